# revision 56
# baseline (speedup 1.0000x reference)
"""AlloLayer forward on 8 TRN2 NeuronCores.

Math: reference computes
    lp   = log_softmax(hs, -1)                      # [B,T,C]
    ex   = exp(lp[..., phone_lab] + alloW)          # [B,T,A]
    sq   = scatter_add(ex, phoneme_lab)             # [B,T,P]
    red  = (sq.sum(-1) - 1) / P
    out  = log(sq - red)

The gather+exp+scatter collapses into a matmul: with
    M[c,p] = sum_{a: phone_lab[a]==c, phoneme_lab[a]==p} exp(alloW[a])
we have sq = softmax(hs) @ M.  Augment M with column 256 = M @ 1 (row sums)
and column 257 = ones, then per 128-row block:
    U = exp(X) @ Maug          (PE, bf16 operands, f32 PSUM accumulate)
    s = U[:,257]  w = U[:,256]
    out = Ln(U[:,0:256] * (1/s) + (s - w)/(P*s))    (one ACT op, per-partition
                                                     scale/bias)

Data-parallel over B*T rows: 16384 rows -> 2048 rows per core, no cross-core
communication; output gathered on host.  Each core's shard is handed to the
device PRE-TRANSPOSED ([C, rows]) so the contraction dim sits on SBUF
partitions: exp output tiles feed the matmul as lhsT directly and no on-chip
transpose is needed.  Maug/labels are tiny and precomputed on host.

Tuned config (CFG below): x cast to fp8e4 on host (quarter input DMA;
rel_l2 3.1e-3 vs the 2e-2 gate), output written bf16 and cast back on host,
one input DMA per 512-row chunk on the SP HWDGE ring (out-DMAs on the same
ring after; scalar-ring DMAs stall the ACT queue and SWDGE costs more), and
the chunk_psum path: per chunk one PSUM tile with 512-col bank-aligned
regions per row-block, one batched DVE reciprocal over the 4 softmax
denominators, one fused (U + b)*inv_s tensor_scalar per block where the
bias column b = (s - w)/P comes straight out of the matmul (maug col P
holds (1 - rowsum(M))/P), one batched Ln, one out-DMA.

PE stage (the measured serializer, ~216ns/MM cold): fp8 DoubleRow matmuls —
exp writes e in fp8e4, maug is fp8e4 scaled by 128 with rows padded to 272
(k-plane stride %16), x shifted by -ln2 on host so e <= 122 stays under the
IEEE-e4m3 240 ceiling; each MM contracts a k-PAIR (K=256), so 32 MMs+LDWs
per chunk-set instead of 64.  The x128 and /2 scales cancel exactly through
the s-column reciprocal.

warm_pe=32: the For_i back-edge idles the PE past the ~3.4us HAM window, so
without countermeasures every iteration's matmuls run at the throttled
1.2 GHz.  32 dummy DoubleRow MMs on the always-resident maug const (no data
deps; overwritten by block 0's start=True MM) run during the input/exp fill
and 16 more cover the Ln/out drain, keeping the PE at 2.4 GHz across the
loop boundary (~2.3us/iter measured).
"""

import os
import numpy as np

import concourse.bass as bass
import concourse.tile as tile
from concourse import bacc, mybir
from concourse import bass_utils

F32 = mybir.dt.float32
F32R = mybir.dt.float32r
BF16 = mybir.dt.bfloat16

N_CORES = 8
B, T, C, A, P = 16, 1024, 512, 4096, 256
ROWS = B * T                      # 16384
R_PER_CORE = ROWS // N_CORES      # 2048
NAUG = P + 3                      # 259: [M | (M@1)/P | ones | ones/P]
BLK = 128
NBLK = R_PER_CORE // BLK          # 16
SUPER = 4                         # row-blocks per DMA superblock
NSUPER = NBLK // SUPER            # 4
KCH = C // BLK                    # 4 contraction chunks


def _pin_act_table(arch):
    """Make natural_log_exp_and_others the only table-set advertising Exp/Ln.

    The compiled NEFF then keeps one resident ACT table set for the whole
    kernel instead of reloading (~1.3us each) on every Exp<->Ln alternation.
    Temporarily mutates the functools-cached dict (set indices unchanged; the
    real HW set genuinely contains both functions, so execution is
    unaffected); returns a restore() closure to undo it after compile.
    """
    from concourse import hw_specs

    tabs = hw_specs.get_activation_tables(arch)
    both = "natural_log_exp_and_others"
    assert both in tabs
    af = mybir.ActivationFunctionType
    assert af.Exp in tabs[both] and af.Ln in tabs[both]
    removed = []
    for name, fns in tabs.items():
        if name != both:
            for f in (af.Exp, af.Ln):
                if f in fns:
                    fns.discard(f)
                    removed.append((fns, f))

    def restore():
        for fns, f in removed:
            fns.add(f)

    return restore


def build_graph(x_bufs=4, e_bufs=3, o_bufs=3, xt_bufs=3, u_bufs=2, out_batch=2,
                bench_iters=0, dma_only=False, compute_only=False,
                out_on_sync=False, bf16_t=False, cast_eng="vector"):
    from contextlib import ExitStack, nullcontext

    nc = bacc.Bacc("TRN2", target_bir_lowering=False, debug=False, num_devices=1)
    _restore_tabs = _pin_act_table(nc.m.arch)
    x_ap = nc.dram_tensor("x", [R_PER_CORE, C], F32, kind="ExternalInput").ap()
    maug_ap = nc.dram_tensor("maug", [KCH, BLK, NAUG], F32, kind="ExternalInput").ap()
    ident_ap = nc.dram_tensor("ident", [BLK, BLK], F32, kind="ExternalInput").ap()
    out_ap = nc.dram_tensor("out", [R_PER_CORE, P], F32, kind="ExternalOutput").ap()

    # Pair schedule: 1-block pairs at both ends — fast pipeline fill at the
    # start, short drain chain at the end.
    PAIRS = [1, 1] + [2] * ((NBLK - 4) // 2) + [1, 1]
    assert sum(PAIRS) == NBLK

    with tile.TileContext(nc) as tc, ExitStack() as ctx:
        const_pool = ctx.enter_context(tc.tile_pool(name="const", bufs=1))
        x_pool = ctx.enter_context(tc.tile_pool(name="xin", bufs=x_bufs))
        e_pool = ctx.enter_context(tc.tile_pool(name="e", bufs=e_bufs))
        o_pool = ctx.enter_context(tc.tile_pool(name="o", bufs=o_bufs))
        s_pool = ctx.enter_context(tc.tile_pool(name="small", bufs=4))
        xt_pool = ctx.enter_context(tc.tile_pool(name="xt", bufs=xt_bufs, space="PSUM"))
        u_pool = ctx.enter_context(tc.tile_pool(name="u", bufs=u_bufs, space="PSUM"))

        # Constants go over SWDGE (Pool) so they don't delay the HWDGE x loads.
        ident_sb = const_pool.tile([BLK, BLK], F32)
        nc.gpsimd.dma_start(ident_sb[:], ident_ap[:, :])
        ident_bf = None
        xb_pool = None
        if bf16_t:
            ident_bf = const_pool.tile([BLK, BLK], BF16)
            nc.gpsimd.dma_start(ident_bf[:], ident_ap[:, :])
            xb_pool = ctx.enter_context(tc.tile_pool(name="xb", bufs=3))
        maug_sb = const_pool.tile([BLK, KCH * NAUG], BF16)
        nc.gpsimd.dma_start(
            maug_sb[:].rearrange("p (k n) -> p k n", n=NAUG),
            maug_ap[:, :, :].rearrange("k p n -> p k n"),
        )

        # bench_iters>0 wraps the whole body in an on-device loop so the
        # per-iteration time can be resolved through the ~1s axon RPC noise.
        # bench_iters < 0: staggered-reset back-edge (cross-iter overlap)
        loop_cm = (
            tc.For_i(0, abs(bench_iters), 1, staggered_reset=bench_iters < 0)
            if bench_iters
            else nullcontext()
        )
        ctx.enter_context(loop_cm)

        row0 = 0
        outs = None
        ob_blocks = 0       # blocks accumulated in current outs tile
        ob_row0 = 0         # first row-block covered by current outs tile
        OBW = out_batch * 2  # max blocks per outs tile

        def flush_outs():
            nonlocal outs, ob_blocks, ob_row0
            if outs is None or ob_blocks == 0:
                return
            # output DMA via SWDGE (Pool) — separate queue from the inputs
            if compute_only:
                outs = None
                ob_blocks = 0
                return
            out_eng = nc.sync if out_on_sync else nc.gpsimd
            out_eng.dma_start(
                out_ap[ob_row0 * BLK:(ob_row0 + ob_blocks) * BLK, :].rearrange(
                    "(b p) c -> p b c", p=BLK
                ),
                outs[:, 0:ob_blocks * P].rearrange("p (b c) -> p b c", c=P),
            )
            outs = None
            ob_blocks = 0

        for pn in PAIRS:
            # input DMA for this pair on the SP HWDGE ring (in-DMAs only, so
            # no out-DMA can head-of-line block the input stream)
            xs = x_pool.tile([BLK, 2 * C], F32, tag="xs")
            if compute_only:
                nc.vector.memset(xs[:, 0:8], 0.0)  # mark tile written
            else:
                nc.sync.dma_start(
                    xs[:, 0:pn * C].rearrange("p (b c) -> p b c", c=C),
                    x_ap[row0 * BLK:(row0 + pn) * BLK, :].rearrange(
                        "(b p) c -> p b c", p=BLK
                    ),
                )
            if outs is None:
                outs = o_pool.tile([BLK, OBW * P], F32, tag="outs")
                ob_row0 = row0
            if dma_only:
                if ob_blocks == 0:
                    nc.vector.memset(outs[:, 0:8], 0.0)  # mark tile written
                ob_blocks += pn
                row0 += pn
                if ob_blocks + 2 > OBW:
                    flush_outs()
                continue
            if bf16_t:
                # cast x to bf16 on an otherwise-idle engine; PE transposes
                # then run at 1 cycle/row (vs 2 for f32) and xt PSUM tiles
                # take 1 bank instead of 2
                xb = xb_pool.tile([BLK, 2 * C], BF16, tag="xb")
                getattr(nc, cast_eng).tensor_copy(xb[:, 0:pn * C], xs[:, 0:pn * C])
                t_src, t_ident, t_dt = xb, ident_bf, BF16
            else:
                t_src, t_ident, t_dt = xs, ident_sb, F32
            xt = xt_pool.tile([BLK, 2 * C], t_dt, tag="xt")  # PSUM
            for bb in range(pn):
                for k in range(KCH):
                    nc.tensor.transpose(
                        xt[:, bb * C + k * BLK:bb * C + (k + 1) * BLK],
                        t_src[:, bb * C + k * BLK:bb * C + (k + 1) * BLK],
                        t_ident[:],
                    )
            e = e_pool.tile([BLK, 2 * C], BF16, tag="e")  # exp(x), [c, r] layout
            nc.scalar.activation(
                e[:, 0:pn * C], xt[:, 0:pn * C],
                mybir.ActivationFunctionType.Exp,
            )
            for bb in range(pn):
                u = u_pool.tile([BLK, NAUG], F32, tag="u")
                for k in range(KCH):
                    nc.tensor.matmul(
                        u[:],
                        e[:, bb * C + k * BLK:bb * C + (k + 1) * BLK],
                        maug_sb[:, k * NAUG:(k + 1) * NAUG],
                        start=(k == 0),
                        stop=(k == KCH - 1),
                    )
                inv_s = s_pool.tile([BLK, 1], F32, tag="inv")
                nc.vector.reciprocal(inv_s[:], u[:, P + 1:P + 2])
                bias_t = s_pool.tile([BLK, 1], F32, tag="bias")
                # bias = (s/P - w/P) * (1/s) = (s - w)/(P*s)
                nc.vector.scalar_tensor_tensor(
                    bias_t[:],
                    u[:, P + 2:P + 3],
                    u[:, P:P + 1],
                    inv_s[:],
                    op0=mybir.AluOpType.subtract,
                    op1=mybir.AluOpType.mult,
                )
                ob = ob_blocks + bb
                nc.scalar.activation(
                    outs[:, ob * P:(ob + 1) * P],
                    u[:, 0:P],
                    mybir.ActivationFunctionType.Ln,
                    bias=bias_t[:],
                    scale=inv_s[:],
                )
            ob_blocks += pn
            row0 += pn
            if ob_blocks + 2 > OBW:
                flush_outs()
        flush_outs()
    try:
        nc.compile()
    finally:
        _restore_tabs()
    return nc


def build_graph_t(x_bufs=3, e_bufs=3, o_bufs=3, u_bufs=6, out_batch=1,
                  rs=512, in_split=2, exp_split=1, out_eng="sync",
                  out_blocks=64, exp_mode="k", maug_bf16=False, ln_batch=1,
                  x_bf16=False, x_fp8=False, out_bf16=False, rsched=None,
                  bench_iters=0, skip_mm=False, skip_dve=False, dma_only=False,
                  compute_only=False, fused_bias=False, chunk_psum=False,
                  in_dual=False, stop_after=None, e_fp8=False, warm_pe=0,
                  recip_pb=False, last_fast=False):
    """Variant taking the per-core x shard PRE-TRANSPOSED on the host:
    x_t[C, R_PER_CORE].  No on-chip transposes: DMA loads [128c, r] tiles
    directly, exp runs on big tiles, PE does only the matmuls.
    """
    from contextlib import ExitStack, nullcontext

    nc = bacc.Bacc("TRN2", target_bir_lowering=False, debug=False, num_devices=1)
    _restore_tabs = _pin_act_table(nc.m.arch)
    x_dt = mybir.dt.float8e4 if x_fp8 else (BF16 if x_bf16 else F32)
    x_ap = nc.dram_tensor("x", [C, R_PER_CORE], x_dt, kind="ExternalInput").ap()
    FP8 = mybir.dt.float8e4
    # fp8 DoubleRow needs the k-plane stride %16 == 0 -> pad maug rows to 272
    NAUGP = 272 if e_fp8 else NAUG
    maug_dt = FP8 if e_fp8 else (BF16 if maug_bf16 else F32)
    maug_ap = nc.dram_tensor("maug", [KCH, BLK, NAUGP], maug_dt, kind="ExternalInput").ap()
    out_dt = BF16 if out_bf16 else F32
    out_ap = nc.dram_tensor("out", [R_PER_CORE, P], out_dt, kind="ExternalOutput").ap()

    if rsched is None:
        rsched = [rs] * (R_PER_CORE // rs)
    assert sum(rsched) == R_PER_CORE

    with tile.TileContext(nc) as tc, ExitStack() as ctx:
        const_pool = ctx.enter_context(tc.tile_pool(name="const", bufs=1))
        x_pool = ctx.enter_context(tc.tile_pool(name="xin", bufs=x_bufs))
        e_pool = ctx.enter_context(tc.tile_pool(name="e", bufs=e_bufs))
        o_pool = ctx.enter_context(tc.tile_pool(name="o", bufs=o_bufs))
        s_pool = ctx.enter_context(tc.tile_pool(name="small", bufs=4))
        v_pool = ctx.enter_context(tc.tile_pool(name="v", bufs=3))
        u_pool = ctx.enter_context(tc.tile_pool(name="u", bufs=u_bufs, space="PSUM"))

        maug_sb = const_pool.tile([BLK, KCH * NAUGP], FP8 if e_fp8 else BF16)
        nc.gpsimd.dma_start(
            maug_sb[:].rearrange("p (k n) -> p k n", n=NAUGP),
            maug_ap[:, :, :].rearrange("k p n -> p k n"),
        )

        loop_cm = (
            tc.For_i(0, abs(bench_iters), 1, staggered_reset=bench_iters < 0)
            if bench_iters
            else nullcontext()
        )
        ctx.enter_context(loop_cm)

        e_dt = FP8 if e_fp8 else BF16
        x_t3 = x_ap.rearrange("(k p) r -> k p r", p=BLK)   # [KCH, 128, R]
        kper = KCH // in_split                              # c-chunks per in-DMA
        RSMAX = max(rsched)
        r0 = 0
        for rs in rsched:
            BPRS = rs // BLK
            # x slice [128, KCH*rs]: c-chunk k occupies cols [k*rs, (k+1)*rs)
            xs = x_pool.tile([BLK, KCH * RSMAX], x_dt, tag="xs")
            if compute_only:
                nc.vector.memset(xs[:, 0:8], 0.0)
            else:
                for d in range(in_split):
                    # in_dual: alternate input DMAs across the two physical
                    # HWDGE rings (qSPDynamicHW / qActDynamicHW)
                    in_eng = nc.scalar if (in_dual and d % 2 == 1) else nc.sync
                    in_eng.dma_start(
                        xs[:, d * kper * rs:(d + 1) * kper * rs].rearrange(
                            "p (k r) -> p k r", r=rs
                        ),
                        x_t3[d * kper:(d + 1) * kper, :, r0:r0 + rs].rearrange(
                            "k p r -> p k r"
                        ),
                    )
            if dma_only:
                outs = o_pool.tile([BLK, (RSMAX // BLK) * P], out_dt, tag="outs")
                nc.vector.memset(outs[:, 0:8], 0.0)
                getattr(nc, out_eng).dma_start(
                    out_ap[r0:r0 + rs, :].rearrange("(b p) c -> p b c", p=BLK),
                    outs[:, 0:BPRS * P].rearrange("p (b c) -> p b c", c=P),
                )
                r0 += rs
                continue
            e = e_pool.tile([BLK, KCH * RSMAX], e_dt, tag="e")
            if exp_mode == "block":
                # one exp per row-block spanning all 4 c-chunks (strided AP):
                # each block's matmuls wait on ONE exp, not all of them
                x3 = xs[:, 0:KCH * rs].rearrange("p (k r) -> p k r", r=rs)
                e3 = e[:, 0:KCH * rs].rearrange("p (k r) -> p k r", r=rs)
                for b in range(rs // BLK):
                    nc.scalar.activation(
                        e3[:, :, b * BLK:(b + 1) * BLK],
                        x3[:, :, b * BLK:(b + 1) * BLK],
                        mybir.ActivationFunctionType.Exp,
                    )
            else:
                estep = KCH * rs // exp_split
                for s in range(exp_split):
                    nc.scalar.activation(
                        e[:, s * estep:(s + 1) * estep],
                        xs[:, s * estep:(s + 1) * estep],
                        mybir.ActivationFunctionType.Exp,
                    )
            outs = o_pool.tile([BLK, (RSMAX // BLK) * P], out_dt, tag="outs")
            if chunk_psum:
                # One PSUM tile spanning the whole chunk (512-col bank-aligned
                # regions per block): batched reciprocal over the 4 s-columns,
                # one fused (U + b)*inv_s per block (needs fused_bias maug),
                # one Ln + one out-DMA per chunk.
                assert fused_bias
                BW = 512  # bank-aligned region per block
                u = u_pool.tile([BLK, (RSMAX // BLK) * BW], F32, tag="u")
                if e_fp8 and warm_pe and r0 == 0:
                    # HAM warm-up: PE re-throttles to 1.2 GHz after >3.4us
                    # idle (the loop back-edge).  Dummy DoubleRow MMs on the
                    # always-resident maug const run during the input-DMA/exp
                    # fill with no data dependency; block 0's real first MM
                    # has start=True so the scribbled region is overwritten.
                    m3w = maug_sb[:].rearrange("p (k n) -> p k n", n=NAUGP)
                    for _ in range(warm_pe):
                        nc.tensor.matmul(
                            u[:, 0:NAUG],
                            m3w[:, 0:2, 0:BLK],
                            m3w[:, 0:2, 0:NAUG],
                            start=True, stop=True,
                            perf_mode=mybir.MatmulPerfMode.DoubleRow,
                        )
                if e_fp8:
                    # fp8 DoubleRow: 2 fp8 weights/cell -> one MM contracts a
                    # k-PAIR (K=256); halves PE work per block (2 MMs vs 4)
                    e3 = e[:, 0:KCH * rs].rearrange("p (k r) -> p k r", r=rs)
                    m3 = maug_sb[:].rearrange("p (k n) -> p k n", n=NAUGP)
                    for b in range(BPRS):
                        for j in range(KCH // 2):
                            nc.tensor.matmul(
                                u[:, b * BW:b * BW + NAUG],
                                e3[:, 2 * j:2 * j + 2, b * BLK:(b + 1) * BLK],
                                m3[:, 2 * j:2 * j + 2, 0:NAUG],
                                start=(j == 0),
                                stop=(j == KCH // 2 - 1),
                                perf_mode=mybir.MatmulPerfMode.DoubleRow,
                            )
                else:
                    for b in range(BPRS):
                        for k in range(KCH):
                            nc.tensor.matmul(
                                u[:, b * BW:b * BW + NAUG],
                                e[:, k * rs + b * BLK:k * rs + (b + 1) * BLK],
                                maug_sb[:, k * NAUG:(k + 1) * NAUG],
                                start=(k == 0),
                                stop=(k == KCH - 1),
                            )
                if stop_after == "mm":
                    r0 += rs
                    continue
                u3 = u[:, 0:BPRS * BW].rearrange("p (b n) -> p b n", n=BW)
                inv_b = s_pool.tile([BLK, RSMAX // BLK], F32, tag="invb")
                is_last = last_fast and (r0 + rs == R_PER_CORE)
                if recip_pb or is_last:
                    # per-block recip: TS(b) depends only on block b's MMs,
                    # not the whole chunk (batched recip reads all s-columns)
                    for b in range(BPRS):
                        nc.vector.reciprocal(
                            inv_b[:, b:b + 1],
                            u[:, b * BW + P + 1:b * BW + P + 2],
                        )
                else:
                    nc.vector.reciprocal(
                        inv_b[:, 0:BPRS].rearrange("p (b o) -> p b o", o=1),
                        u3[:, :, P + 1:P + 2],
                    )
                v = v_pool.tile([BLK, (RSMAX // BLK) * P], F32, tag="v")
                for b in range(BPRS):
                    nc.vector.tensor_scalar(
                        v[:, b * P:(b + 1) * P],
                        u[:, b * BW:b * BW + P],
                        u[:, b * BW + P:b * BW + P + 1],
                        inv_b[:, b:b + 1],
                        op0=mybir.AluOpType.add,
                        op1=mybir.AluOpType.mult,
                    )
                if stop_after == "dve":
                    r0 += rs
                    continue
                # last chunk: halve Ln + out-DMA so the drain chain after the
                # final tensor_scalar is Ln(2 blocks) + 128KB store, not
                # Ln(4) + 256KB
                nhalf = 2 if (is_last and BPRS % 2 == 0) else 1
                hb = BPRS // nhalf
                for h in range(nhalf):
                    nc.scalar.activation(
                        outs[:, h * hb * P:(h + 1) * hb * P],
                        v[:, h * hb * P:(h + 1) * hb * P],
                        mybir.ActivationFunctionType.Ln,
                    )
                    if stop_after == "ln":
                        continue
                    getattr(nc, out_eng).dma_start(
                        out_ap[r0 + h * hb * BLK:r0 + (h + 1) * hb * BLK, :]
                        .rearrange("(b p) c -> p b c", p=BLK),
                        outs[:, h * hb * P:(h + 1) * hb * P]
                        .rearrange("p (b c) -> p b c", c=P),
                    )
                r0 += rs
                continue
            if skip_mm:
                nc.vector.memset(outs[:, 0:8], 0.0)
            for b in range(BPRS if not skip_mm else 0):
                u = u_pool.tile([BLK, NAUG], F32, tag="u")
                for k in range(KCH):
                    nc.tensor.matmul(
                        u[:],
                        e[:, k * rs + b * BLK:k * rs + (b + 1) * BLK],
                        maug_sb[:, k * NAUG:(k + 1) * NAUG],
                        start=(k == 0),
                        stop=(k == KCH - 1),
                    )
                if skip_dve:
                    nc.scalar.activation(
                        outs[:, b * P:(b + 1) * P],
                        u[:, 0:P],
                        mybir.ActivationFunctionType.Ln,
                        bias=0.0,
                        scale=1.0,
                    )
                    continue
                inv_s = s_pool.tile([BLK, 1], F32, tag="inv")
                nc.vector.reciprocal(inv_s[:], u[:, P + 1:P + 2])
                bias_t = s_pool.tile([BLK, 1], F32, tag="bias")
                if ln_batch > 1:
                    # normalize on DVE (per-partition scalars), then one Ln
                    # per ln_batch blocks — fewer serial ACT instructions
                    if b % ln_batch == 0:
                        v = v_pool.tile([BLK, ln_batch * P], F32, tag="v")
                    if fused_bias:
                        # maug col P already holds (1 - rowsum(M))/P, so
                        # U[:,P] = (s - w)/P directly — no small op needed
                        bias_ap = u[:, P:P + 1]
                    else:
                        # bias2 = s/P - w/P = (s - w)/P
                        nc.vector.tensor_scalar(
                            bias_t[:],
                            u[:, P + 2:P + 3],
                            u[:, P:P + 1],
                            None,
                            op0=mybir.AluOpType.subtract,
                        )
                        bias_ap = bias_t[:]
                    # V = (U + bias2) * inv_s
                    nc.vector.tensor_scalar(
                        v[:, (b % ln_batch) * P:(b % ln_batch + 1) * P],
                        u[:, 0:P],
                        bias_ap,
                        inv_s[:],
                        op0=mybir.AluOpType.add,
                        op1=mybir.AluOpType.mult,
                    )
                    if (b + 1) % ln_batch == 0 or b == BPRS - 1:
                        g0 = (b // ln_batch) * ln_batch
                        ng = b - g0 + 1
                        nc.scalar.activation(
                            outs[:, g0 * P:(g0 + ng) * P],
                            v[:, 0:ng * P],
                            mybir.ActivationFunctionType.Ln,
                        )
                else:
                    if fused_bias:
                        # U[:,P] = (s - w)/P; bias = U[:,P] * (1/s)
                        nc.vector.tensor_scalar(
                            bias_t[:],
                            u[:, P:P + 1],
                            inv_s[:],
                            None,
                            op0=mybir.AluOpType.mult,
                        )
                    else:
                        # bias = (s/P - w/P) * (1/s) = (s - w)/(P*s)
                        nc.vector.scalar_tensor_tensor(
                            bias_t[:],
                            u[:, P + 2:P + 3],
                            u[:, P:P + 1],
                            inv_s[:],
                            op0=mybir.AluOpType.subtract,
                            op1=mybir.AluOpType.mult,
                        )
                    nc.scalar.activation(
                        outs[:, b * P:(b + 1) * P],
                        u[:, 0:P],
                        mybir.ActivationFunctionType.Ln,
                        bias=bias_t[:],
                        scale=inv_s[:],
                    )
                if (b + 1) % out_blocks == 0 or b == BPRS - 1:
                    b0 = (b // out_blocks) * out_blocks
                    nb = b - b0 + 1
                    getattr(nc, out_eng).dma_start(
                        out_ap[r0 + b0 * BLK:r0 + (b0 + nb) * BLK, :].rearrange(
                            "(b p) c -> p b c", p=BLK
                        ),
                        outs[:, b0 * P:(b0 + nb) * P].rearrange(
                            "p (b c) -> p b c", c=P
                        ),
                    )
            r0 += rs
        if chunk_psum and e_fp8 and warm_pe and stop_after is None and not dma_only:
            # tail warm-up: keep PE busy through the Ln/out-DMA drain; writes
            # the last chunk's block-0 u region AFTER its DVE reads (WAR)
            m3w = maug_sb[:].rearrange("p (k n) -> p k n", n=NAUGP)
            for _ in range(warm_pe // 2):
                nc.tensor.matmul(
                    u[:, 0:NAUG],
                    m3w[:, 0:2, 0:BLK],
                    m3w[:, 0:2, 0:NAUG],
                    start=True, stop=True,
                    perf_mode=mybir.MatmulPerfMode.DoubleRow,
                )
        if chunk_psum and stop_after in ("mm", "dve", "ln"):
            # probe modes skip the real out-DMAs; bind the output tensor with
            # one tiny store on the otherwise-idle Pool ring
            dummy = o_pool.tile([BLK, 8], out_dt, tag="dummy")
            nc.vector.memset(dummy[:], 0.0)
            nc.gpsimd.dma_start(out_ap[0:BLK, 0:8], dummy[:])
    try:
        nc.compile()
    finally:
        _restore_tabs()
    return nc


def build_graph_p(x_bufs=5, e_bufs=4, o_bufs=3, v_bufs=3, u_bufs=2,
                  rsched=None, exp_la=2, x_bf16=True, x_fp8=False,
                  out_bf16=True, out_eng="gpsimd", out_last_sync=True,
                  recip_pb=False, ln_split=1, bench_iters=0):
    """Software-pipelined variant.

    The ACT engine queue is strict FIFO, so in naive program order Ln(j)
    blocks exp(j+1) while it waits out the PE->DVE round trip of chunk j —
    serializing ~7us per chunk.  Here exp(j+exp_la) is issued BEFORE Ln(j)
    so ACT streams continuously.  Input DMAs get the SP HWDGE ring to
    themselves; output goes via SWDGE (Pool) so no out-DMA can block either
    the input ring or the ACT queue.  Per chunk: one PSUM tile spanning all
    blocks (512-col bank-aligned regions), one batched reciprocal, one fused
    (U + b)*inv_s per block (maug col P holds (1-rowsum(M))/P), one Ln, one
    out-DMA.
    """
    from contextlib import ExitStack, nullcontext

    nc = bacc.Bacc("TRN2", target_bir_lowering=False, debug=False, num_devices=1)
    _restore_tabs = _pin_act_table(nc.m.arch)
    x_dt = mybir.dt.float8e4 if x_fp8 else (BF16 if x_bf16 else F32)
    out_dt = BF16 if out_bf16 else F32
    x_ap = nc.dram_tensor("x", [C, R_PER_CORE], x_dt, kind="ExternalInput").ap()
    maug_ap = nc.dram_tensor("maug", [KCH, BLK, NAUG], F32, kind="ExternalInput").ap()
    out_ap = nc.dram_tensor("out", [R_PER_CORE, P], out_dt, kind="ExternalOutput").ap()

    if rsched is None:
        rsched = [512] * (R_PER_CORE // 512)
    assert sum(rsched) == R_PER_CORE
    n = len(rsched)
    r0s = [sum(rsched[:j]) for j in range(n)]
    RSMAX = max(rsched)
    BW = 512  # PSUM bank-aligned region per row-block

    with tile.TileContext(nc) as tc, ExitStack() as ctx:
        const_pool = ctx.enter_context(tc.tile_pool(name="const", bufs=1))
        x_pool = ctx.enter_context(tc.tile_pool(name="xin", bufs=x_bufs))
        e_pool = ctx.enter_context(tc.tile_pool(name="e", bufs=e_bufs))
        o_pool = ctx.enter_context(tc.tile_pool(name="o", bufs=o_bufs))
        s_pool = ctx.enter_context(tc.tile_pool(name="small", bufs=4))
        v_pool = ctx.enter_context(tc.tile_pool(name="v", bufs=v_bufs))
        u_pool = ctx.enter_context(tc.tile_pool(name="u", bufs=u_bufs, space="PSUM"))

        maug_sb = const_pool.tile([BLK, KCH * NAUG], BF16)
        nc.gpsimd.dma_start(
            maug_sb[:].rearrange("p (k n) -> p k n", n=NAUG),
            maug_ap[:, :, :].rearrange("k p n -> p k n"),
        )

        loop_cm = (
            tc.For_i(0, abs(bench_iters), 1, staggered_reset=bench_iters < 0)
            if bench_iters
            else nullcontext()
        )
        ctx.enter_context(loop_cm)

        x_t3 = x_ap.rearrange("(k p) r -> k p r", p=BLK)   # [KCH, 128, R]
        xs_t = [None] * n
        e_t = [None] * n

        def issue_in(j):
            rs = rsched[j]
            xs = x_pool.tile([BLK, KCH * RSMAX], x_dt, tag="xs")
            nc.sync.dma_start(
                xs[:, 0:KCH * rs].rearrange("p (k r) -> p k r", r=rs),
                x_t3[:, :, r0s[j]:r0s[j] + rs].rearrange("k p r -> p k r"),
            )
            xs_t[j] = xs

        def issue_exp(j):
            rs = rsched[j]
            e = e_pool.tile([BLK, KCH * RSMAX], BF16, tag="e")
            nc.scalar.activation(
                e[:, 0:KCH * rs], xs_t[j][:, 0:KCH * rs],
                mybir.ActivationFunctionType.Exp,
            )
            e_t[j] = e

        def issue_rest(j):
            rs = rsched[j]
            BPRS = rs // BLK
            e = e_t[j]
            u = u_pool.tile([BLK, (RSMAX // BLK) * BW], F32, tag="u")
            for b in range(BPRS):
                for k in range(KCH):
                    nc.tensor.matmul(
                        u[:, b * BW:b * BW + NAUG],
                        e[:, k * rs + b * BLK:k * rs + (b + 1) * BLK],
                        maug_sb[:, k * NAUG:(k + 1) * NAUG],
                        start=(k == 0),
                        stop=(k == KCH - 1),
                    )
            u3 = u[:, 0:BPRS * BW].rearrange("p (b n) -> p b n", n=BW)
            inv_b = s_pool.tile([BLK, RSMAX // BLK], F32, tag="invb")
            if recip_pb:
                # per-block recip: v(b) depends only on block b's matmuls,
                # not the whole chunk — shorter dependency chain
                for b in range(BPRS):
                    nc.vector.reciprocal(
                        inv_b[:, b:b + 1],
                        u[:, b * BW + P + 1:b * BW + P + 2],
                    )
            else:
                nc.vector.reciprocal(
                    inv_b[:, 0:BPRS].rearrange("p (b o) -> p b o", o=1),
                    u3[:, :, P + 1:P + 2],
                )
            v = v_pool.tile([BLK, (RSMAX // BLK) * P], F32, tag="v")
            for b in range(BPRS):
                nc.vector.tensor_scalar(
                    v[:, b * P:(b + 1) * P],
                    u[:, b * BW:b * BW + P],
                    u[:, b * BW + P:b * BW + P + 1],
                    inv_b[:, b:b + 1],
                    op0=mybir.AluOpType.add,
                    op1=mybir.AluOpType.mult,
                )
            outs = o_pool.tile([BLK, (RSMAX // BLK) * P], out_dt, tag="outs")
            gsz = max(1, BPRS // ln_split)
            for g0 in range(0, BPRS, gsz):
                ng = min(gsz, BPRS - g0)
                nc.scalar.activation(
                    outs[:, g0 * P:(g0 + ng) * P], v[:, g0 * P:(g0 + ng) * P],
                    mybir.ActivationFunctionType.Ln,
                )
            eng = nc.sync if (out_last_sync and j == n - 1) else getattr(nc, out_eng)
            eng.dma_start(
                out_ap[r0s[j]:r0s[j] + rs, :].rearrange("(b p) c -> p b c", p=BLK),
                outs[:, 0:BPRS * P].rearrange("p (b c) -> p b c", c=P),
            )

        # prologue: stay exp_la chunks ahead on exp, exp_la+1 on input DMA
        in_la = exp_la + 1
        for j in range(min(in_la, n)):
            issue_in(j)
            if j < exp_la:
                issue_exp(j)
        for j in range(n):
            if j + in_la < n:
                issue_in(j + in_la)
            if j + exp_la < n:
                issue_exp(j + exp_la)
            issue_rest(j)
    try:
        nc.compile()
    finally:
        _restore_tabs()
    return nc


def make_maug(alloW, phone_arc_labels, phoneme_arc_labels, fused_bias=False,
              scale=1.0, pad_to=NAUG):
    alloW = np.asarray(alloW, dtype=np.float64).reshape(-1)
    phone = np.asarray(phone_arc_labels).astype(np.int64).reshape(-1)
    phoneme = np.asarray(phoneme_arc_labels).astype(np.int64).reshape(-1)
    M = np.zeros((C, P), dtype=np.float64)
    np.add.at(M, (phone, phoneme), np.exp(alloW))
    maug = np.zeros((C, pad_to), dtype=np.float64)
    maug[:, :P] = M
    if fused_bias:
        # U[:,256] = (s - w)/P in one matmul column: sum_c e_c (1 - rowsum)/P
        maug[:, P] = (1.0 - M.sum(axis=1)) / P
    else:
        maug[:, P] = M.sum(axis=1) / P      # U[:,256] = w/P
    maug[:, P + 1] = 1.0                    # U[:,257] = s  (softmax denom)
    maug[:, P + 2] = 1.0 / P                # U[:,258] = s/P
    # uniform scale (e.g. 256 for fp8): lifts the tiny bias column out of the
    # fp8 flush-to-zero range; cancels exactly through the reciprocal since
    # the s column scales identically
    maug *= scale
    return maug.astype(np.float32).reshape(KCH, BLK, pad_to)


_NC = None

# Chosen build config — shared by _get_nc and test.py so timing measures the
# exact graph that kernel() runs.  builder="p" -> build_graph_p (software-
# pipelined); anything else -> build_graph_t.
CFG = dict(builder="t", x_fp8=True, out_bf16=True, fused_bias=True,
           chunk_psum=True, e_fp8=True, warm_pe=32, rs=512, in_split=1,
           exp_split=1, out_eng="sync", o_bufs=5, u_bufs=2, x_bufs=4,
           e_bufs=3)


def build_from_cfg(cfg=None, **extra):
    cfg = dict(CFG if cfg is None else cfg)
    cfg.update(extra)
    builder = cfg.pop("builder", "t")
    if builder == "p":
        return build_graph_p(**cfg)
    return build_graph_t(**cfg)


def cfg_fused_bias(cfg):
    return cfg.get("builder") == "p" or cfg.get("fused_bias", False)


def maug_kwargs(cfg):
    kw = dict(fused_bias=cfg_fused_bias(cfg))
    if cfg.get("e_fp8"):
        # TRN2 fp8e4 is IEEE e4m3: max 240. Scale 128 keeps the s column
        # (=scale) and M entries in range while lifting the bias column out
        # of flush-to-zero; exp is shifted by -ln2 so e <= 122.
        kw.update(scale=128.0, pad_to=272)
    return kw


def _x_host_dtype(cfg):
    import ml_dtypes

    if cfg.get("x_fp8"):
        return ml_dtypes.float8_e4m3
    if cfg.get("x_bf16"):
        return ml_dtypes.bfloat16
    return np.float32


def make_in_maps(hs, maug, cfg=CFG):
    """Shard + pre-transpose + cast the [ROWS, C] f32 hs for the device."""
    import ml_dtypes

    xdt = _x_host_dtype(cfg)
    if cfg.get("e_fp8"):
        # shift x by -ln2 so e = exp(x)/2 stays under the fp8e4 (IEEE e4m3)
        # 240 ceiling; the factor cancels via the s column / reciprocal
        hs = hs - np.float32(0.6931471805599453)
        maug = maug.astype(ml_dtypes.float8_e4m3)
    hs = hs.astype(xdt) if xdt is not np.float32 else hs
    return [
        {
            "x": np.ascontiguousarray(hs[i * R_PER_CORE:(i + 1) * R_PER_CORE].T),
            "maug": maug,
        }
        for i in range(N_CORES)
    ]


def _get_nc():
    global _NC
    if _NC is None:
        _NC = build_from_cfg()
    return _NC


def run(hs_pad, alloW, phone_arc_labels, phoneme_arc_labels, n_phonemes, trace=False):
    import time

    assert int(n_phonemes) == P
    hs = np.ascontiguousarray(np.asarray(hs_pad, dtype=np.float32)).reshape(ROWS, C)
    maug = make_maug(alloW, phone_arc_labels, phoneme_arc_labels,
                     **maug_kwargs(CFG))
    # data-parallel shard over rows; shards handed to the device pre-transposed
    # ([C, r] layout) so the contraction dim lands on SBUF partitions with no
    # on-chip transpose
    in_maps = make_in_maps(hs, maug, CFG)
    nc = _get_nc()
    last_err = None
    for attempt in range(7):
        try:
            res = bass_utils.run_bass_kernel_spmd(
                nc, in_maps, core_ids=list(range(N_CORES)), trace=trace
            )
            break
        except Exception as e:  # transient NRT exec-unit errors recover on retry
            last_err = e
            time.sleep(min(2.0 * (attempt + 1), 10.0))
    else:
        raise last_err
    out = np.concatenate(
        [np.asarray(res.results[i]["out"]).astype(np.float32) for i in range(N_CORES)],
        axis=0,
    )
    return out.reshape(B, T, P), res


def kernel(hs_pad, alloW, phone_arc_labels, phoneme_arc_labels, n_phonemes):
    out, _ = run(hs_pad, alloW, phone_arc_labels, phoneme_arc_labels, n_phonemes)
    return out



# revision 58
# speedup vs baseline: 1.1297x; 1.1297x over previous
"""AlloLayer forward on 8 TRN2 NeuronCores.

Math: reference computes
    lp   = log_softmax(hs, -1)                      # [B,T,C]
    ex   = exp(lp[..., phone_lab] + alloW)          # [B,T,A]
    sq   = scatter_add(ex, phoneme_lab)             # [B,T,P]
    red  = (sq.sum(-1) - 1) / P
    out  = log(sq - red)

The gather+exp+scatter collapses into a matmul: with
    M[c,p] = sum_{a: phone_lab[a]==c, phoneme_lab[a]==p} exp(alloW[a])
we have sq = softmax(hs) @ M.  Augment M with column 256 = M @ 1 (row sums)
and column 257 = ones, then per 128-row block:
    U = exp(X) @ Maug          (PE, bf16 operands, f32 PSUM accumulate)
    s = U[:,257]  w = U[:,256]
    out = Ln(U[:,0:256] * (1/s) + (s - w)/(P*s))    (one ACT op, per-partition
                                                     scale/bias)

Data-parallel over B*T rows: 16384 rows -> 2048 rows per core, no cross-core
communication; output gathered on host.  Each core's shard is handed to the
device PRE-TRANSPOSED ([C, rows]) so the contraction dim sits on SBUF
partitions: exp output tiles feed the matmul as lhsT directly and no on-chip
transpose is needed.  Maug/labels are tiny and precomputed on host.

Tuned config (CFG below): x cast to fp8e4 on host (quarter input DMA;
rel_l2 3.1e-3 vs the 2e-2 gate), output written bf16 and cast back on host,
one input DMA per 512-row chunk on the SP HWDGE ring (out-DMAs on the same
ring after; scalar-ring DMAs stall the ACT queue and SWDGE costs more), and
the chunk_psum path: per chunk one PSUM tile with 512-col bank-aligned
regions per row-block, one batched DVE reciprocal over the 4 softmax
denominators, one fused (U + b)*inv_s tensor_scalar per block where the
bias column b = (s - w)/P comes straight out of the matmul (maug col P
holds (1 - rowsum(M))/P), one batched Ln, one out-DMA.

PE stage (the measured serializer, ~216ns/MM cold): fp8 DoubleRow matmuls —
exp writes e in fp8e4, maug is fp8e4 scaled by 128 with rows padded to 272
(k-plane stride %16), x shifted by -ln2 on host so e <= 122 stays under the
IEEE-e4m3 240 ceiling; each MM contracts a k-PAIR (K=256), so 32 MMs+LDWs
per chunk-set instead of 64.  The x128 and /2 scales cancel exactly through
the s-column reciprocal.

warm_pe=32: the For_i back-edge idles the PE past the ~3.4us HAM window, so
without countermeasures every iteration's matmuls run at the throttled
1.2 GHz.  32 dummy DoubleRow MMs on the always-resident maug const (no data
deps; overwritten by block 0's start=True MM) run during the input/exp fill
and 16 more cover the Ln/out drain, keeping the PE at 2.4 GHz across the
loop boundary (~2.3us/iter measured).
"""

import os
import numpy as np

import concourse.bass as bass
import concourse.tile as tile
from concourse import bacc, mybir
from concourse import bass_utils

F32 = mybir.dt.float32
F32R = mybir.dt.float32r
BF16 = mybir.dt.bfloat16

N_CORES = 8
B, T, C, A, P = 16, 1024, 512, 4096, 256
ROWS = B * T                      # 16384
R_PER_CORE = ROWS // N_CORES      # 2048
NAUG = P + 3                      # 259: [M | (M@1)/P | ones | ones/P]
BLK = 128
NBLK = R_PER_CORE // BLK          # 16
SUPER = 4                         # row-blocks per DMA superblock
NSUPER = NBLK // SUPER            # 4
KCH = C // BLK                    # 4 contraction chunks


def _pin_act_table(arch):
    """Make natural_log_exp_and_others the only table-set advertising Exp/Ln.

    The compiled NEFF then keeps one resident ACT table set for the whole
    kernel instead of reloading (~1.3us each) on every Exp<->Ln alternation.
    Temporarily mutates the functools-cached dict (set indices unchanged; the
    real HW set genuinely contains both functions, so execution is
    unaffected); returns a restore() closure to undo it after compile.
    """
    from concourse import hw_specs

    tabs = hw_specs.get_activation_tables(arch)
    both = "natural_log_exp_and_others"
    assert both in tabs
    af = mybir.ActivationFunctionType
    assert af.Exp in tabs[both] and af.Ln in tabs[both]
    removed = []
    for name, fns in tabs.items():
        if name != both:
            for f in (af.Exp, af.Ln):
                if f in fns:
                    fns.discard(f)
                    removed.append((fns, f))

    def restore():
        for fns, f in removed:
            fns.add(f)

    return restore


def build_graph(x_bufs=4, e_bufs=3, o_bufs=3, xt_bufs=3, u_bufs=2, out_batch=2,
                bench_iters=0, dma_only=False, compute_only=False,
                out_on_sync=False, bf16_t=False, cast_eng="vector"):
    from contextlib import ExitStack, nullcontext

    nc = bacc.Bacc("TRN2", target_bir_lowering=False, debug=False, num_devices=1)
    _restore_tabs = _pin_act_table(nc.m.arch)
    x_ap = nc.dram_tensor("x", [R_PER_CORE, C], F32, kind="ExternalInput").ap()
    maug_ap = nc.dram_tensor("maug", [KCH, BLK, NAUG], F32, kind="ExternalInput").ap()
    ident_ap = nc.dram_tensor("ident", [BLK, BLK], F32, kind="ExternalInput").ap()
    out_ap = nc.dram_tensor("out", [R_PER_CORE, P], F32, kind="ExternalOutput").ap()

    # Pair schedule: 1-block pairs at both ends — fast pipeline fill at the
    # start, short drain chain at the end.
    PAIRS = [1, 1] + [2] * ((NBLK - 4) // 2) + [1, 1]
    assert sum(PAIRS) == NBLK

    with tile.TileContext(nc) as tc, ExitStack() as ctx:
        const_pool = ctx.enter_context(tc.tile_pool(name="const", bufs=1))
        x_pool = ctx.enter_context(tc.tile_pool(name="xin", bufs=x_bufs))
        e_pool = ctx.enter_context(tc.tile_pool(name="e", bufs=e_bufs))
        o_pool = ctx.enter_context(tc.tile_pool(name="o", bufs=o_bufs))
        s_pool = ctx.enter_context(tc.tile_pool(name="small", bufs=4))
        xt_pool = ctx.enter_context(tc.tile_pool(name="xt", bufs=xt_bufs, space="PSUM"))
        u_pool = ctx.enter_context(tc.tile_pool(name="u", bufs=u_bufs, space="PSUM"))

        # Constants go over SWDGE (Pool) so they don't delay the HWDGE x loads.
        ident_sb = const_pool.tile([BLK, BLK], F32)
        nc.gpsimd.dma_start(ident_sb[:], ident_ap[:, :])
        ident_bf = None
        xb_pool = None
        if bf16_t:
            ident_bf = const_pool.tile([BLK, BLK], BF16)
            nc.gpsimd.dma_start(ident_bf[:], ident_ap[:, :])
            xb_pool = ctx.enter_context(tc.tile_pool(name="xb", bufs=3))
        maug_sb = const_pool.tile([BLK, KCH * NAUG], BF16)
        nc.gpsimd.dma_start(
            maug_sb[:].rearrange("p (k n) -> p k n", n=NAUG),
            maug_ap[:, :, :].rearrange("k p n -> p k n"),
        )

        # bench_iters>0 wraps the whole body in an on-device loop so the
        # per-iteration time can be resolved through the ~1s axon RPC noise.
        # bench_iters < 0: staggered-reset back-edge (cross-iter overlap)
        loop_cm = (
            tc.For_i(0, abs(bench_iters), 1, staggered_reset=bench_iters < 0)
            if bench_iters
            else nullcontext()
        )
        ctx.enter_context(loop_cm)

        row0 = 0
        outs = None
        ob_blocks = 0       # blocks accumulated in current outs tile
        ob_row0 = 0         # first row-block covered by current outs tile
        OBW = out_batch * 2  # max blocks per outs tile

        def flush_outs():
            nonlocal outs, ob_blocks, ob_row0
            if outs is None or ob_blocks == 0:
                return
            # output DMA via SWDGE (Pool) — separate queue from the inputs
            if compute_only:
                outs = None
                ob_blocks = 0
                return
            out_eng = nc.sync if out_on_sync else nc.gpsimd
            out_eng.dma_start(
                out_ap[ob_row0 * BLK:(ob_row0 + ob_blocks) * BLK, :].rearrange(
                    "(b p) c -> p b c", p=BLK
                ),
                outs[:, 0:ob_blocks * P].rearrange("p (b c) -> p b c", c=P),
            )
            outs = None
            ob_blocks = 0

        for pn in PAIRS:
            # input DMA for this pair on the SP HWDGE ring (in-DMAs only, so
            # no out-DMA can head-of-line block the input stream)
            xs = x_pool.tile([BLK, 2 * C], F32, tag="xs")
            if compute_only:
                nc.vector.memset(xs[:, 0:8], 0.0)  # mark tile written
            else:
                nc.sync.dma_start(
                    xs[:, 0:pn * C].rearrange("p (b c) -> p b c", c=C),
                    x_ap[row0 * BLK:(row0 + pn) * BLK, :].rearrange(
                        "(b p) c -> p b c", p=BLK
                    ),
                )
            if outs is None:
                outs = o_pool.tile([BLK, OBW * P], F32, tag="outs")
                ob_row0 = row0
            if dma_only:
                if ob_blocks == 0:
                    nc.vector.memset(outs[:, 0:8], 0.0)  # mark tile written
                ob_blocks += pn
                row0 += pn
                if ob_blocks + 2 > OBW:
                    flush_outs()
                continue
            if bf16_t:
                # cast x to bf16 on an otherwise-idle engine; PE transposes
                # then run at 1 cycle/row (vs 2 for f32) and xt PSUM tiles
                # take 1 bank instead of 2
                xb = xb_pool.tile([BLK, 2 * C], BF16, tag="xb")
                getattr(nc, cast_eng).tensor_copy(xb[:, 0:pn * C], xs[:, 0:pn * C])
                t_src, t_ident, t_dt = xb, ident_bf, BF16
            else:
                t_src, t_ident, t_dt = xs, ident_sb, F32
            xt = xt_pool.tile([BLK, 2 * C], t_dt, tag="xt")  # PSUM
            for bb in range(pn):
                for k in range(KCH):
                    nc.tensor.transpose(
                        xt[:, bb * C + k * BLK:bb * C + (k + 1) * BLK],
                        t_src[:, bb * C + k * BLK:bb * C + (k + 1) * BLK],
                        t_ident[:],
                    )
            e = e_pool.tile([BLK, 2 * C], BF16, tag="e")  # exp(x), [c, r] layout
            nc.scalar.activation(
                e[:, 0:pn * C], xt[:, 0:pn * C],
                mybir.ActivationFunctionType.Exp,
            )
            for bb in range(pn):
                u = u_pool.tile([BLK, NAUG], F32, tag="u")
                for k in range(KCH):
                    nc.tensor.matmul(
                        u[:],
                        e[:, bb * C + k * BLK:bb * C + (k + 1) * BLK],
                        maug_sb[:, k * NAUG:(k + 1) * NAUG],
                        start=(k == 0),
                        stop=(k == KCH - 1),
                    )
                inv_s = s_pool.tile([BLK, 1], F32, tag="inv")
                nc.vector.reciprocal(inv_s[:], u[:, P + 1:P + 2])
                bias_t = s_pool.tile([BLK, 1], F32, tag="bias")
                # bias = (s/P - w/P) * (1/s) = (s - w)/(P*s)
                nc.vector.scalar_tensor_tensor(
                    bias_t[:],
                    u[:, P + 2:P + 3],
                    u[:, P:P + 1],
                    inv_s[:],
                    op0=mybir.AluOpType.subtract,
                    op1=mybir.AluOpType.mult,
                )
                ob = ob_blocks + bb
                nc.scalar.activation(
                    outs[:, ob * P:(ob + 1) * P],
                    u[:, 0:P],
                    mybir.ActivationFunctionType.Ln,
                    bias=bias_t[:],
                    scale=inv_s[:],
                )
            ob_blocks += pn
            row0 += pn
            if ob_blocks + 2 > OBW:
                flush_outs()
        flush_outs()
    try:
        nc.compile()
    finally:
        _restore_tabs()
    return nc


def build_graph_t(x_bufs=3, e_bufs=3, o_bufs=3, u_bufs=6, out_batch=1,
                  rs=512, in_split=2, exp_split=1, out_eng="sync",
                  out_blocks=64, exp_mode="k", maug_bf16=False, ln_batch=1,
                  x_bf16=False, x_fp8=False, out_bf16=False, rsched=None,
                  bench_iters=0, skip_mm=False, skip_dve=False, dma_only=False,
                  compute_only=False, fused_bias=False, chunk_psum=False,
                  in_dual=False, stop_after=None, e_fp8=False, warm_pe=0,
                  recip_pb=False, last_fast=False, out_last_sync=False):
    """Variant taking the per-core x shard PRE-TRANSPOSED on the host:
    x_t[C, R_PER_CORE].  No on-chip transposes: DMA loads [128c, r] tiles
    directly, exp runs on big tiles, PE does only the matmuls.
    """
    from contextlib import ExitStack, nullcontext

    nc = bacc.Bacc("TRN2", target_bir_lowering=False, debug=False, num_devices=1)
    _restore_tabs = _pin_act_table(nc.m.arch)
    x_dt = mybir.dt.float8e4 if x_fp8 else (BF16 if x_bf16 else F32)
    x_ap = nc.dram_tensor("x", [C, R_PER_CORE], x_dt, kind="ExternalInput").ap()
    FP8 = mybir.dt.float8e4
    # fp8 DoubleRow needs the k-plane stride %16 == 0 -> pad maug rows to 272
    NAUGP = 272 if e_fp8 else NAUG
    maug_dt = FP8 if e_fp8 else (BF16 if maug_bf16 else F32)
    maug_ap = nc.dram_tensor("maug", [KCH, BLK, NAUGP], maug_dt, kind="ExternalInput").ap()
    out_dt = BF16 if out_bf16 else F32
    out_ap = nc.dram_tensor("out", [R_PER_CORE, P], out_dt, kind="ExternalOutput").ap()

    if rsched is None:
        rsched = [rs] * (R_PER_CORE // rs)
    assert sum(rsched) == R_PER_CORE

    with tile.TileContext(nc) as tc, ExitStack() as ctx:
        const_pool = ctx.enter_context(tc.tile_pool(name="const", bufs=1))
        x_pool = ctx.enter_context(tc.tile_pool(name="xin", bufs=x_bufs))
        e_pool = ctx.enter_context(tc.tile_pool(name="e", bufs=e_bufs))
        o_pool = ctx.enter_context(tc.tile_pool(name="o", bufs=o_bufs))
        s_pool = ctx.enter_context(tc.tile_pool(name="small", bufs=4))
        v_pool = ctx.enter_context(tc.tile_pool(name="v", bufs=3))
        u_pool = ctx.enter_context(tc.tile_pool(name="u", bufs=u_bufs, space="PSUM"))

        maug_sb = const_pool.tile([BLK, KCH * NAUGP], FP8 if e_fp8 else BF16)
        nc.gpsimd.dma_start(
            maug_sb[:].rearrange("p (k n) -> p k n", n=NAUGP),
            maug_ap[:, :, :].rearrange("k p n -> p k n"),
        )

        loop_cm = (
            tc.For_i(0, abs(bench_iters), 1, staggered_reset=bench_iters < 0)
            if bench_iters
            else nullcontext()
        )
        ctx.enter_context(loop_cm)

        e_dt = FP8 if e_fp8 else BF16
        x_t3 = x_ap.rearrange("(k p) r -> k p r", p=BLK)   # [KCH, 128, R]
        kper = KCH // in_split                              # c-chunks per in-DMA
        RSMAX = max(rsched)
        r0 = 0
        for rs in rsched:
            BPRS = rs // BLK
            # x slice [128, KCH*rs]: c-chunk k occupies cols [k*rs, (k+1)*rs)
            xs = x_pool.tile([BLK, KCH * RSMAX], x_dt, tag="xs")
            if compute_only:
                nc.vector.memset(xs[:, 0:8], 0.0)
            else:
                for d in range(in_split):
                    # in_dual: alternate input DMAs across the two physical
                    # HWDGE rings (qSPDynamicHW / qActDynamicHW)
                    in_eng = nc.scalar if (in_dual and d % 2 == 1) else nc.sync
                    in_eng.dma_start(
                        xs[:, d * kper * rs:(d + 1) * kper * rs].rearrange(
                            "p (k r) -> p k r", r=rs
                        ),
                        x_t3[d * kper:(d + 1) * kper, :, r0:r0 + rs].rearrange(
                            "k p r -> p k r"
                        ),
                    )
            if dma_only:
                outs = o_pool.tile([BLK, (RSMAX // BLK) * P], out_dt, tag="outs")
                nc.vector.memset(outs[:, 0:8], 0.0)
                getattr(nc, out_eng).dma_start(
                    out_ap[r0:r0 + rs, :].rearrange("(b p) c -> p b c", p=BLK),
                    outs[:, 0:BPRS * P].rearrange("p (b c) -> p b c", c=P),
                )
                r0 += rs
                continue
            e = e_pool.tile([BLK, KCH * RSMAX], e_dt, tag="e")
            if exp_mode == "block":
                # one exp per row-block spanning all 4 c-chunks (strided AP):
                # each block's matmuls wait on ONE exp, not all of them
                x3 = xs[:, 0:KCH * rs].rearrange("p (k r) -> p k r", r=rs)
                e3 = e[:, 0:KCH * rs].rearrange("p (k r) -> p k r", r=rs)
                for b in range(rs // BLK):
                    nc.scalar.activation(
                        e3[:, :, b * BLK:(b + 1) * BLK],
                        x3[:, :, b * BLK:(b + 1) * BLK],
                        mybir.ActivationFunctionType.Exp,
                    )
            else:
                estep = KCH * rs // exp_split
                for s in range(exp_split):
                    nc.scalar.activation(
                        e[:, s * estep:(s + 1) * estep],
                        xs[:, s * estep:(s + 1) * estep],
                        mybir.ActivationFunctionType.Exp,
                    )
            outs = o_pool.tile([BLK, (RSMAX // BLK) * P], out_dt, tag="outs")
            if chunk_psum:
                # One PSUM tile spanning the whole chunk (512-col bank-aligned
                # regions per block): batched reciprocal over the 4 s-columns,
                # one fused (U + b)*inv_s per block (needs fused_bias maug),
                # one Ln + one out-DMA per chunk.
                assert fused_bias
                BW = 512  # bank-aligned region per block
                u = u_pool.tile([BLK, (RSMAX // BLK) * BW], F32, tag="u")
                if e_fp8 and warm_pe and r0 == 0:
                    # HAM warm-up: PE re-throttles to 1.2 GHz after >3.4us
                    # idle (the loop back-edge).  Dummy DoubleRow MMs on the
                    # always-resident maug const run during the input-DMA/exp
                    # fill with no data dependency; block 0's real first MM
                    # has start=True so the scribbled region is overwritten.
                    m3w = maug_sb[:].rearrange("p (k n) -> p k n", n=NAUGP)
                    for _ in range(warm_pe):
                        nc.tensor.matmul(
                            u[:, 0:NAUG],
                            m3w[:, 0:2, 0:BLK],
                            m3w[:, 0:2, 0:NAUG],
                            start=True, stop=True,
                            perf_mode=mybir.MatmulPerfMode.DoubleRow,
                        )
                if e_fp8:
                    # fp8 DoubleRow: 2 fp8 weights/cell -> one MM contracts a
                    # k-PAIR (K=256); halves PE work per block (2 MMs vs 4)
                    e3 = e[:, 0:KCH * rs].rearrange("p (k r) -> p k r", r=rs)
                    m3 = maug_sb[:].rearrange("p (k n) -> p k n", n=NAUGP)
                    for b in range(BPRS):
                        for j in range(KCH // 2):
                            nc.tensor.matmul(
                                u[:, b * BW:b * BW + NAUG],
                                e3[:, 2 * j:2 * j + 2, b * BLK:(b + 1) * BLK],
                                m3[:, 2 * j:2 * j + 2, 0:NAUG],
                                start=(j == 0),
                                stop=(j == KCH // 2 - 1),
                                perf_mode=mybir.MatmulPerfMode.DoubleRow,
                            )
                else:
                    for b in range(BPRS):
                        for k in range(KCH):
                            nc.tensor.matmul(
                                u[:, b * BW:b * BW + NAUG],
                                e[:, k * rs + b * BLK:k * rs + (b + 1) * BLK],
                                maug_sb[:, k * NAUG:(k + 1) * NAUG],
                                start=(k == 0),
                                stop=(k == KCH - 1),
                            )
                if stop_after == "mm":
                    r0 += rs
                    continue
                u3 = u[:, 0:BPRS * BW].rearrange("p (b n) -> p b n", n=BW)
                inv_b = s_pool.tile([BLK, RSMAX // BLK], F32, tag="invb")
                is_last = last_fast and (r0 + rs == R_PER_CORE)
                if recip_pb or is_last:
                    # per-block recip: TS(b) depends only on block b's MMs,
                    # not the whole chunk (batched recip reads all s-columns)
                    for b in range(BPRS):
                        nc.vector.reciprocal(
                            inv_b[:, b:b + 1],
                            u[:, b * BW + P + 1:b * BW + P + 2],
                        )
                else:
                    nc.vector.reciprocal(
                        inv_b[:, 0:BPRS].rearrange("p (b o) -> p b o", o=1),
                        u3[:, :, P + 1:P + 2],
                    )
                v = v_pool.tile([BLK, (RSMAX // BLK) * P], F32, tag="v")
                for b in range(BPRS):
                    nc.vector.tensor_scalar(
                        v[:, b * P:(b + 1) * P],
                        u[:, b * BW:b * BW + P],
                        u[:, b * BW + P:b * BW + P + 1],
                        inv_b[:, b:b + 1],
                        op0=mybir.AluOpType.add,
                        op1=mybir.AluOpType.mult,
                    )
                if stop_after == "dve":
                    r0 += rs
                    continue
                # last chunk: halve Ln + out-DMA so the drain chain after the
                # final tensor_scalar is Ln(2 blocks) + 128KB store, not
                # Ln(4) + 256KB
                nhalf = 2 if (is_last and BPRS % 2 == 0) else 1
                hb = BPRS // nhalf
                # keep early outs off the input ring; the final store goes on
                # sync HWDGE (the in-DMAs have long drained by then) to avoid
                # SWDGE's ~2us fixed cost in the drain
                o_eng = (nc.sync if (out_last_sync and r0 + rs == R_PER_CORE)
                         else getattr(nc, out_eng))
                for h in range(nhalf):
                    nc.scalar.activation(
                        outs[:, h * hb * P:(h + 1) * hb * P],
                        v[:, h * hb * P:(h + 1) * hb * P],
                        mybir.ActivationFunctionType.Ln,
                    )
                    if stop_after == "ln":
                        continue
                    o_eng.dma_start(
                        out_ap[r0 + h * hb * BLK:r0 + (h + 1) * hb * BLK, :]
                        .rearrange("(b p) c -> p b c", p=BLK),
                        outs[:, h * hb * P:(h + 1) * hb * P]
                        .rearrange("p (b c) -> p b c", c=P),
                    )
                r0 += rs
                continue
            if skip_mm:
                nc.vector.memset(outs[:, 0:8], 0.0)
            for b in range(BPRS if not skip_mm else 0):
                u = u_pool.tile([BLK, NAUG], F32, tag="u")
                for k in range(KCH):
                    nc.tensor.matmul(
                        u[:],
                        e[:, k * rs + b * BLK:k * rs + (b + 1) * BLK],
                        maug_sb[:, k * NAUG:(k + 1) * NAUG],
                        start=(k == 0),
                        stop=(k == KCH - 1),
                    )
                if skip_dve:
                    nc.scalar.activation(
                        outs[:, b * P:(b + 1) * P],
                        u[:, 0:P],
                        mybir.ActivationFunctionType.Ln,
                        bias=0.0,
                        scale=1.0,
                    )
                    continue
                inv_s = s_pool.tile([BLK, 1], F32, tag="inv")
                nc.vector.reciprocal(inv_s[:], u[:, P + 1:P + 2])
                bias_t = s_pool.tile([BLK, 1], F32, tag="bias")
                if ln_batch > 1:
                    # normalize on DVE (per-partition scalars), then one Ln
                    # per ln_batch blocks — fewer serial ACT instructions
                    if b % ln_batch == 0:
                        v = v_pool.tile([BLK, ln_batch * P], F32, tag="v")
                    if fused_bias:
                        # maug col P already holds (1 - rowsum(M))/P, so
                        # U[:,P] = (s - w)/P directly — no small op needed
                        bias_ap = u[:, P:P + 1]
                    else:
                        # bias2 = s/P - w/P = (s - w)/P
                        nc.vector.tensor_scalar(
                            bias_t[:],
                            u[:, P + 2:P + 3],
                            u[:, P:P + 1],
                            None,
                            op0=mybir.AluOpType.subtract,
                        )
                        bias_ap = bias_t[:]
                    # V = (U + bias2) * inv_s
                    nc.vector.tensor_scalar(
                        v[:, (b % ln_batch) * P:(b % ln_batch + 1) * P],
                        u[:, 0:P],
                        bias_ap,
                        inv_s[:],
                        op0=mybir.AluOpType.add,
                        op1=mybir.AluOpType.mult,
                    )
                    if (b + 1) % ln_batch == 0 or b == BPRS - 1:
                        g0 = (b // ln_batch) * ln_batch
                        ng = b - g0 + 1
                        nc.scalar.activation(
                            outs[:, g0 * P:(g0 + ng) * P],
                            v[:, 0:ng * P],
                            mybir.ActivationFunctionType.Ln,
                        )
                else:
                    if fused_bias:
                        # U[:,P] = (s - w)/P; bias = U[:,P] * (1/s)
                        nc.vector.tensor_scalar(
                            bias_t[:],
                            u[:, P:P + 1],
                            inv_s[:],
                            None,
                            op0=mybir.AluOpType.mult,
                        )
                    else:
                        # bias = (s/P - w/P) * (1/s) = (s - w)/(P*s)
                        nc.vector.scalar_tensor_tensor(
                            bias_t[:],
                            u[:, P + 2:P + 3],
                            u[:, P:P + 1],
                            inv_s[:],
                            op0=mybir.AluOpType.subtract,
                            op1=mybir.AluOpType.mult,
                        )
                    nc.scalar.activation(
                        outs[:, b * P:(b + 1) * P],
                        u[:, 0:P],
                        mybir.ActivationFunctionType.Ln,
                        bias=bias_t[:],
                        scale=inv_s[:],
                    )
                if (b + 1) % out_blocks == 0 or b == BPRS - 1:
                    b0 = (b // out_blocks) * out_blocks
                    nb = b - b0 + 1
                    getattr(nc, out_eng).dma_start(
                        out_ap[r0 + b0 * BLK:r0 + (b0 + nb) * BLK, :].rearrange(
                            "(b p) c -> p b c", p=BLK
                        ),
                        outs[:, b0 * P:(b0 + nb) * P].rearrange(
                            "p (b c) -> p b c", c=P
                        ),
                    )
            r0 += rs
        if chunk_psum and e_fp8 and warm_pe and stop_after is None and not dma_only:
            # tail warm-up: keep PE busy through the Ln/out-DMA drain; writes
            # the last chunk's block-0 u region AFTER its DVE reads (WAR)
            m3w = maug_sb[:].rearrange("p (k n) -> p k n", n=NAUGP)
            for _ in range(warm_pe // 2):
                nc.tensor.matmul(
                    u[:, 0:NAUG],
                    m3w[:, 0:2, 0:BLK],
                    m3w[:, 0:2, 0:NAUG],
                    start=True, stop=True,
                    perf_mode=mybir.MatmulPerfMode.DoubleRow,
                )
        if chunk_psum and stop_after in ("mm", "dve", "ln"):
            # probe modes skip the real out-DMAs; bind the output tensor with
            # one tiny store on the otherwise-idle Pool ring
            dummy = o_pool.tile([BLK, 8], out_dt, tag="dummy")
            nc.vector.memset(dummy[:], 0.0)
            nc.gpsimd.dma_start(out_ap[0:BLK, 0:8], dummy[:])
    try:
        nc.compile()
    finally:
        _restore_tabs()
    return nc


def build_graph_p(x_bufs=5, e_bufs=4, o_bufs=3, v_bufs=3, u_bufs=2,
                  rsched=None, exp_la=2, x_bf16=True, x_fp8=False,
                  out_bf16=True, out_eng="gpsimd", out_last_sync=True,
                  recip_pb=False, ln_split=1, bench_iters=0):
    """Software-pipelined variant.

    The ACT engine queue is strict FIFO, so in naive program order Ln(j)
    blocks exp(j+1) while it waits out the PE->DVE round trip of chunk j —
    serializing ~7us per chunk.  Here exp(j+exp_la) is issued BEFORE Ln(j)
    so ACT streams continuously.  Input DMAs get the SP HWDGE ring to
    themselves; output goes via SWDGE (Pool) so no out-DMA can block either
    the input ring or the ACT queue.  Per chunk: one PSUM tile spanning all
    blocks (512-col bank-aligned regions), one batched reciprocal, one fused
    (U + b)*inv_s per block (maug col P holds (1-rowsum(M))/P), one Ln, one
    out-DMA.
    """
    from contextlib import ExitStack, nullcontext

    nc = bacc.Bacc("TRN2", target_bir_lowering=False, debug=False, num_devices=1)
    _restore_tabs = _pin_act_table(nc.m.arch)
    x_dt = mybir.dt.float8e4 if x_fp8 else (BF16 if x_bf16 else F32)
    out_dt = BF16 if out_bf16 else F32
    x_ap = nc.dram_tensor("x", [C, R_PER_CORE], x_dt, kind="ExternalInput").ap()
    maug_ap = nc.dram_tensor("maug", [KCH, BLK, NAUG], F32, kind="ExternalInput").ap()
    out_ap = nc.dram_tensor("out", [R_PER_CORE, P], out_dt, kind="ExternalOutput").ap()

    if rsched is None:
        rsched = [512] * (R_PER_CORE // 512)
    assert sum(rsched) == R_PER_CORE
    n = len(rsched)
    r0s = [sum(rsched[:j]) for j in range(n)]
    RSMAX = max(rsched)
    BW = 512  # PSUM bank-aligned region per row-block

    with tile.TileContext(nc) as tc, ExitStack() as ctx:
        const_pool = ctx.enter_context(tc.tile_pool(name="const", bufs=1))
        x_pool = ctx.enter_context(tc.tile_pool(name="xin", bufs=x_bufs))
        e_pool = ctx.enter_context(tc.tile_pool(name="e", bufs=e_bufs))
        o_pool = ctx.enter_context(tc.tile_pool(name="o", bufs=o_bufs))
        s_pool = ctx.enter_context(tc.tile_pool(name="small", bufs=4))
        v_pool = ctx.enter_context(tc.tile_pool(name="v", bufs=v_bufs))
        u_pool = ctx.enter_context(tc.tile_pool(name="u", bufs=u_bufs, space="PSUM"))

        maug_sb = const_pool.tile([BLK, KCH * NAUG], BF16)
        nc.gpsimd.dma_start(
            maug_sb[:].rearrange("p (k n) -> p k n", n=NAUG),
            maug_ap[:, :, :].rearrange("k p n -> p k n"),
        )

        loop_cm = (
            tc.For_i(0, abs(bench_iters), 1, staggered_reset=bench_iters < 0)
            if bench_iters
            else nullcontext()
        )
        ctx.enter_context(loop_cm)

        x_t3 = x_ap.rearrange("(k p) r -> k p r", p=BLK)   # [KCH, 128, R]
        xs_t = [None] * n
        e_t = [None] * n

        def issue_in(j):
            rs = rsched[j]
            xs = x_pool.tile([BLK, KCH * RSMAX], x_dt, tag="xs")
            nc.sync.dma_start(
                xs[:, 0:KCH * rs].rearrange("p (k r) -> p k r", r=rs),
                x_t3[:, :, r0s[j]:r0s[j] + rs].rearrange("k p r -> p k r"),
            )
            xs_t[j] = xs

        def issue_exp(j):
            rs = rsched[j]
            e = e_pool.tile([BLK, KCH * RSMAX], BF16, tag="e")
            nc.scalar.activation(
                e[:, 0:KCH * rs], xs_t[j][:, 0:KCH * rs],
                mybir.ActivationFunctionType.Exp,
            )
            e_t[j] = e

        def issue_rest(j):
            rs = rsched[j]
            BPRS = rs // BLK
            e = e_t[j]
            u = u_pool.tile([BLK, (RSMAX // BLK) * BW], F32, tag="u")
            for b in range(BPRS):
                for k in range(KCH):
                    nc.tensor.matmul(
                        u[:, b * BW:b * BW + NAUG],
                        e[:, k * rs + b * BLK:k * rs + (b + 1) * BLK],
                        maug_sb[:, k * NAUG:(k + 1) * NAUG],
                        start=(k == 0),
                        stop=(k == KCH - 1),
                    )
            u3 = u[:, 0:BPRS * BW].rearrange("p (b n) -> p b n", n=BW)
            inv_b = s_pool.tile([BLK, RSMAX // BLK], F32, tag="invb")
            if recip_pb:
                # per-block recip: v(b) depends only on block b's matmuls,
                # not the whole chunk — shorter dependency chain
                for b in range(BPRS):
                    nc.vector.reciprocal(
                        inv_b[:, b:b + 1],
                        u[:, b * BW + P + 1:b * BW + P + 2],
                    )
            else:
                nc.vector.reciprocal(
                    inv_b[:, 0:BPRS].rearrange("p (b o) -> p b o", o=1),
                    u3[:, :, P + 1:P + 2],
                )
            v = v_pool.tile([BLK, (RSMAX // BLK) * P], F32, tag="v")
            for b in range(BPRS):
                nc.vector.tensor_scalar(
                    v[:, b * P:(b + 1) * P],
                    u[:, b * BW:b * BW + P],
                    u[:, b * BW + P:b * BW + P + 1],
                    inv_b[:, b:b + 1],
                    op0=mybir.AluOpType.add,
                    op1=mybir.AluOpType.mult,
                )
            outs = o_pool.tile([BLK, (RSMAX // BLK) * P], out_dt, tag="outs")
            gsz = max(1, BPRS // ln_split)
            for g0 in range(0, BPRS, gsz):
                ng = min(gsz, BPRS - g0)
                nc.scalar.activation(
                    outs[:, g0 * P:(g0 + ng) * P], v[:, g0 * P:(g0 + ng) * P],
                    mybir.ActivationFunctionType.Ln,
                )
            eng = nc.sync if (out_last_sync and j == n - 1) else getattr(nc, out_eng)
            eng.dma_start(
                out_ap[r0s[j]:r0s[j] + rs, :].rearrange("(b p) c -> p b c", p=BLK),
                outs[:, 0:BPRS * P].rearrange("p (b c) -> p b c", c=P),
            )

        # prologue: stay exp_la chunks ahead on exp, exp_la+1 on input DMA
        in_la = exp_la + 1
        for j in range(min(in_la, n)):
            issue_in(j)
            if j < exp_la:
                issue_exp(j)
        for j in range(n):
            if j + in_la < n:
                issue_in(j + in_la)
            if j + exp_la < n:
                issue_exp(j + exp_la)
            issue_rest(j)
    try:
        nc.compile()
    finally:
        _restore_tabs()
    return nc


def make_maug(alloW, phone_arc_labels, phoneme_arc_labels, fused_bias=False,
              scale=1.0, pad_to=NAUG):
    alloW = np.asarray(alloW, dtype=np.float64).reshape(-1)
    phone = np.asarray(phone_arc_labels).astype(np.int64).reshape(-1)
    phoneme = np.asarray(phoneme_arc_labels).astype(np.int64).reshape(-1)
    M = np.zeros((C, P), dtype=np.float64)
    np.add.at(M, (phone, phoneme), np.exp(alloW))
    maug = np.zeros((C, pad_to), dtype=np.float64)
    maug[:, :P] = M
    if fused_bias:
        # U[:,256] = (s - w)/P in one matmul column: sum_c e_c (1 - rowsum)/P
        maug[:, P] = (1.0 - M.sum(axis=1)) / P
    else:
        maug[:, P] = M.sum(axis=1) / P      # U[:,256] = w/P
    maug[:, P + 1] = 1.0                    # U[:,257] = s  (softmax denom)
    maug[:, P + 2] = 1.0 / P                # U[:,258] = s/P
    # uniform scale (e.g. 256 for fp8): lifts the tiny bias column out of the
    # fp8 flush-to-zero range; cancels exactly through the reciprocal since
    # the s column scales identically
    maug *= scale
    return maug.astype(np.float32).reshape(KCH, BLK, pad_to)


_NC = None

# Chosen build config — shared by _get_nc and test.py so timing measures the
# exact graph that kernel() runs.  builder="p" -> build_graph_p (software-
# pipelined); anything else -> build_graph_t.
CFG = dict(builder="t", x_fp8=True, out_bf16=True, fused_bias=True,
           chunk_psum=True, e_fp8=True, warm_pe=32, rs=512, in_split=1,
           exp_split=1, out_eng="sync", o_bufs=5, u_bufs=2, x_bufs=4,
           e_bufs=3)


def build_from_cfg(cfg=None, **extra):
    cfg = dict(CFG if cfg is None else cfg)
    cfg.update(extra)
    builder = cfg.pop("builder", "t")
    if builder == "p":
        return build_graph_p(**cfg)
    return build_graph_t(**cfg)


def cfg_fused_bias(cfg):
    return cfg.get("builder") == "p" or cfg.get("fused_bias", False)


def maug_kwargs(cfg):
    kw = dict(fused_bias=cfg_fused_bias(cfg))
    if cfg.get("e_fp8"):
        # TRN2 fp8e4 is IEEE e4m3: max 240. Scale 128 keeps the s column
        # (=scale) and M entries in range while lifting the bias column out
        # of flush-to-zero; exp is shifted by -ln2 so e <= 122.
        kw.update(scale=128.0, pad_to=272)
    return kw


def _x_host_dtype(cfg):
    import ml_dtypes

    if cfg.get("x_fp8"):
        return ml_dtypes.float8_e4m3
    if cfg.get("x_bf16"):
        return ml_dtypes.bfloat16
    return np.float32


def make_in_maps(hs, maug, cfg=CFG):
    """Shard + pre-transpose + cast the [ROWS, C] f32 hs for the device."""
    import ml_dtypes

    xdt = _x_host_dtype(cfg)
    if cfg.get("e_fp8"):
        # shift x by -ln2 so e = exp(x)/2 stays under the fp8e4 (IEEE e4m3)
        # 240 ceiling; the factor cancels via the s column / reciprocal
        hs = hs - np.float32(0.6931471805599453)
        maug = maug.astype(ml_dtypes.float8_e4m3)
    hs = hs.astype(xdt) if xdt is not np.float32 else hs
    return [
        {
            "x": np.ascontiguousarray(hs[i * R_PER_CORE:(i + 1) * R_PER_CORE].T),
            "maug": maug,
        }
        for i in range(N_CORES)
    ]


def _get_nc():
    global _NC
    if _NC is None:
        _NC = build_from_cfg()
    return _NC


def run(hs_pad, alloW, phone_arc_labels, phoneme_arc_labels, n_phonemes, trace=False):
    import time

    assert int(n_phonemes) == P
    hs = np.ascontiguousarray(np.asarray(hs_pad, dtype=np.float32)).reshape(ROWS, C)
    maug = make_maug(alloW, phone_arc_labels, phoneme_arc_labels,
                     **maug_kwargs(CFG))
    # data-parallel shard over rows; shards handed to the device pre-transposed
    # ([C, r] layout) so the contraction dim lands on SBUF partitions with no
    # on-chip transpose
    in_maps = make_in_maps(hs, maug, CFG)
    nc = _get_nc()
    last_err = None
    for attempt in range(7):
        try:
            res = bass_utils.run_bass_kernel_spmd(
                nc, in_maps, core_ids=list(range(N_CORES)), trace=trace
            )
            break
        except Exception as e:  # transient NRT exec-unit errors recover on retry
            last_err = e
            time.sleep(min(2.0 * (attempt + 1), 10.0))
    else:
        raise last_err
    out = np.concatenate(
        [np.asarray(res.results[i]["out"]).astype(np.float32) for i in range(N_CORES)],
        axis=0,
    )
    return out.reshape(B, T, P), res


def kernel(hs_pad, alloW, phone_arc_labels, phoneme_arc_labels, n_phonemes):
    out, _ = run(hs_pad, alloW, phone_arc_labels, phoneme_arc_labels, n_phonemes)
    return out

